# revision 19
# baseline (speedup 1.0000x reference)
"""Trainium2 Bass kernel for the exp-kernel multivariate Hawkes process
log-likelihood (B=8, N=2048, D=10).

Strategy
--------
Data-parallel over batch: core b computes batch row b fully on-chip and
returns one scalar; the host gathers the 8 scalars.

Per core the O(N^2) pairwise interaction is restructured into a chunked
O(N*D^2) algorithm (chunk size C=128 = partition count). Over (r,m) =
(receiver, trigger) type pairs (RM=100), with per-chunk reference times
ts_k:

  W[j,(r,m)]  = [e_j==m] * exp( b[r,m] (t_j - ts_k))
  P           = inclusive prefix of W over j within the chunk
                (PE matmul with upper-triangular ones)
  S_k[(r,m)]  = sum_{j < chunk k} exp(-b[r,m](ts_k - t_j))
                (inter-chunk state; affine scan over chunks)
  lam_i       = musub[e_i] + sum_{r,m} [e_i==r] exp(-b(t_i-ts)) ab[r,m] (P+S)[i,(r,m)]

The inclusive prefix counts the self pair j==i contributing exactly
ab[e_i,e_i]; host-precomputed musub = mu - diag(ab) cancels it.

The inter-chunk recurrence S_{k+1} = d_k*(S_k + Wsum_k) is ONE
`tensor_tensor_scan` in transposed layout [100,16]; per-chunk column
sums come from N=1 matmuls, and S is broadcast into PSUM with K=1
matmuls against a flattened S row.

The integral term uses the same masking trick with transposed tables:
  neg_ev_j = sum_m onehot[j,m] (asumT[m] - sum_d aT[m,d] exp(bT[m,d](t_j-T)))

Precision: exp arguments and all accumulations are fp32; post-exp
values, 0/1 masks, and matmul operands are bf16 (DVE 2x mode + PE
single-pass). Host-side work is limited to O(D^2) parameter softplus,
O(N) reshapes and the 16 chunk reference times.
"""
import numpy as np
from contextlib import ExitStack

import ml_dtypes
import concourse.bass as bass
import concourse.mybir as mybir
import concourse.tile as tile
from concourse import bacc
from concourse.bass_utils import run_bass_kernel_spmd

f32 = mybir.dt.float32
bf16 = mybir.dt.bfloat16
AL = mybir.AluOpType
AF = mybir.ActivationFunctionType
AX = mybir.AxisListType

P = 128          # partitions == chunk size
KC = 16          # number of chunks
D = 10           # event types
RM = D * D       # (receiver, trigger) pairs
N = P * KC       # 2048 events per batch row
B = 8            # batch == cores
NG = 4           # chunk groups (4 chunks per PSUM bank)

# packed DRAM inputs: name -> (shape, dtype)
INPUTS = {
    "pack_f32": ((P, 285), f32),    # t(16) e(16) tstart(16) bflat(100)
                                    # musub(10) asumT(10) ident(100)
                                    # bdtb(16) negconst(1)
    "pack_bf": ((P, 588), bf16),    # triu(128) abflat(100) aTflat(100)
                                    # bTflat(100) iota(160)
}


def _body(ctx: ExitStack, tc, ins, out_ap, Tval: float):
    nc = tc.nc
    cpool = ctx.enter_context(tc.tile_pool(name="cpool", bufs=1))
    wpool = ctx.enter_context(tc.tile_pool(name="wpool", bufs=1))
    spool = ctx.enter_context(tc.tile_pool(name="spool", bufs=1))
    pp = ctx.enter_context(tc.tile_pool(name="pp", bufs=1, space="PSUM"))
    ps = ctx.enter_context(tc.tile_pool(name="ps", bufs=1, space="PSUM"))

    # ---- load packed inputs on two parallel DMA queues ----
    pf = cpool.tile([P, 285], f32, tag="pf")
    nc.sync.dma_start(out=pf[:], in_=ins["pack_f32"])
    pb = cpool.tile([P, 588], bf16, tag="pb")
    nc.sync.dma_start(out=pb[:], in_=ins["pack_bf"])

    t128 = pf[:, 0:16]
    e128 = pf[:, 16:32]
    tstart = pf[:, 32:48]
    bflat = pf[:, 48:148].rearrange("p (r m) -> p r m", r=D)
    musub = pf[:, 148:158]
    asumT = pf[:, 158:168]
    ident = pf[0:RM, 168:268]
    bdtb = pf[0:RM, 268:284]
    negconst = pf[0:1, 284:285]
    triu = pb[:, 0:128]
    abflat = pb[:, 128:228].rearrange("p (r m) -> p r m", r=D)
    aTflat = pb[:, 228:328].rearrange("p (m d) -> p m d", m=D)
    bTflat = pb[:, 328:428].rearrange("p (m d) -> p m d", m=D)
    iota10 = pb[:, 428:588]

    # ---- constants ----
    ones_row = cpool.tile([1, P], bf16, tag="ones_row")
    nc.vector.memset(ones_row[:], 1.0)
    ones_col = cpool.tile([P, 1], f32, tag="ones_col")
    nc.vector.memset(ones_col[:], 1.0)
    ones_col_bf = cpool.tile([P, 1], bf16, tag="ones_col_bf")
    nc.vector.memset(ones_col_bf[:], 1.0)

    # ---- per-event scalars ----
    onehot = wpool.tile([P, KC, D], bf16, tag="onehot")
    nc.vector.tensor_tensor(
        out=onehot[:], in0=e128.unsqueeze(2).broadcast_to([P, KC, D]),
        in1=iota10.rearrange("p (k d) -> p k d", k=KC), op=AL.is_equal)
    trel = wpool.tile([P, KC], f32, tag="trel")
    nc.vector.tensor_tensor(out=trel[:], in0=t128, in1=tstart, op=AL.subtract)
    tau2 = wpool.tile([P, KC], f32, tag="tau2")
    nc.vector.tensor_scalar_add(tau2[:], t128, -Tval)

    # ---- positive-part exp pipeline (per group, so the PE starts early) ----
    argW = wpool.tile([P, KC, D, D], f32, tag="argW")
    expW = wpool.tile([P, KC, D, D], bf16, tag="expW")
    for g in range(NG):
        gs = slice(4 * g, 4 * (g + 1))
        nc.vector.tensor_tensor(
            out=argW[:, gs],
            in0=trel[:, gs].unsqueeze(2).unsqueeze(3)
                .broadcast_to([P, 4, D, D]),
            in1=bflat.unsqueeze(1).broadcast_to([P, 4, D, D]),
            op=AL.mult)
        nc.scalar.activation(expW[:, gs], argW[:, gs], AF.Exp)
    expU = wpool.tile([P, KC, D, D], bf16, tag="expU")
    nc.scalar.activation(expU[:], argW[:], AF.Exp, scale=-1.0)

    # W = expW * onehot[m]; all-bf16 SBUF => DVE 2x mode, per group for
    # PE overlap
    W = wpool.tile([P, KC, D, D], bf16, tag="W")
    for g in range(NG):
        gs = slice(4 * g, 4 * (g + 1))
        nc.vector.tensor_tensor(
            out=W[:, gs], in0=expW[:, gs],
            in1=onehot[:, gs].unsqueeze(2).broadcast_to([P, 4, D, D]),
            op=AL.mult)
    # expUab = exp(-argW) * ab;  U2ab = expUab * onehot[r]  (all-bf16, 2x)
    expUab = wpool.tile([P, KC, D, D], bf16, tag="expUab")
    nc.vector.tensor_tensor(
        out=expUab[:], in0=expU[:],
        in1=abflat.unsqueeze(1).broadcast_to([P, KC, D, D]), op=AL.mult)
    # musub_ev[i,k] = (mu - diag(ab))[e_i]
    msm = wpool.tile([P, KC, D], f32, tag="msm")
    nc.vector.tensor_tensor(
        out=msm[:], in0=onehot[:],
        in1=musub.unsqueeze(1).broadcast_to([P, KC, D]), op=AL.mult)
    musub_ev = wpool.tile([P, KC], f32, tag="musub_ev")
    nc.vector.tensor_reduce(out=musub_ev[:], in_=msm[:], axis=AX.X, op=AL.add)

    # ---- negative-part arg on GPSIMD (idle engine, overlaps DVE) ----
    argN = wpool.tile([P, KC, D, D], f32, tag="argN")
    nc.gpsimd.tensor_tensor(
        out=argN[:],
        in0=tau2[:].unsqueeze(2).unsqueeze(3).broadcast_to([P, KC, D, D]),
        in1=bTflat.unsqueeze(1).broadcast_to([P, KC, D, D]),
        op=AL.mult)

    # ---- PE phase ----
    Pg = [pp.tile([P, 4, D, D], f32, tag=f"Pg{g}", name=f"Pg{g}")
          for g in range(NG)]
    wsumc = ps.tile([RM, KC], f32, tag="wsumc")
    for k in range(KC):
        nc.tensor.matmul(wsumc[:, k:k + 1],
                         W[:, k].rearrange("p r m -> p (r m)"),
                         ones_col_bf[:], start=True, stop=True)
    for g in range(NG):
        nc.tensor.matmul(Pg[g][:],
                         triu,
                         W[:, 4 * g:4 * (g + 1)].rearrange(
                             "p c r m -> p (c r m)"),
                         start=True, stop=False)

    # ---- inter-chunk affine scan (transposed layout [100,16]) ----
    decayT = spool.tile([RM, KC], f32, tag="decayT")
    nc.scalar.activation(decayT[:], bdtb, AF.Exp, scale=-1.0)
    V = spool.tile([RM, KC], f32, tag="V")
    nc.vector.tensor_tensor(out=V[:], in0=decayT[:], in1=wsumc[:], op=AL.mult)
    SCOL = spool.tile([RM, KC], f32, tag="SCOL")
    nc.vector.tensor_tensor_scan(SCOL[:], decayT[:], V[:], initial=0.0,
                                 op0=AL.mult, op1=AL.add)
    # SCOL[:, t] = S_{t+1}; transpose and flatten to a partition-0 row
    # (matmul operands must be quadrant-aligned), block 0 = S_0 = 0
    stp = ps.tile([KC, RM], f32, tag="stp")
    nc.tensor.transpose(stp[:], SCOL[:], ident)
    srows = spool.tile([KC, RM], bf16, tag="srows")
    nc.vector.tensor_copy(out=srows[:], in_=stp[:])
    sflat = spool.tile([1, KC * RM], bf16, tag="sflat")
    nc.vector.memset(sflat[:, :RM], 0.0)
    nc.sync.dma_start(out=sflat[:, RM:], in_=srows[:KC - 1, :])

    # batched S inject: one K=1 matmul per group broadcasts S_k to all rows
    for g in range(NG):
        nc.tensor.matmul(Pg[g][:], ones_row[:],
                         sflat[:, g * 4 * RM:(g + 1) * 4 * RM],
                         start=False, stop=True)

    # ---- positive part: lam via fused multiply-reduce per chunk ----
    # lam[:,k] = musub_ev[:,k] + sum_rm U2ab[:,k,rm] * (P+S)[:,k,rm]
    lamr = wpool.tile([P, KC], f32, tag="lamr")
    PM = wpool.tile([P, KC, D, D], bf16, tag="PM")
    G2 = wpool.tile([P, KC, D, D], bf16, tag="G2")
    for g in range(NG):
        gs = slice(4 * g, 4 * (g + 1))
        nc.vector.tensor_tensor(
            out=PM[:, gs], in0=Pg[g][:],
            in1=onehot[:, gs].unsqueeze(3).broadcast_to([P, 4, D, D]),
            op=AL.mult)
        nc.vector.tensor_tensor(out=G2[:, gs], in0=PM[:, gs],
                                in1=expUab[:, gs], op=AL.mult)
        nc.vector.tensor_reduce(
            out=lamr[:, gs],
            in_=G2[:, gs].rearrange("p c r m -> p c (r m)"),
            axis=AX.X, op=AL.add)
    lam = wpool.tile([P, KC], f32, tag="lam")
    nc.vector.tensor_tensor(out=lam[:], in0=lamr[:], in1=musub_ev[:],
                            op=AL.add)
    loglam = wpool.tile([P, KC], f32, tag="loglam")
    nc.scalar.activation(loglam[:], lam[:], AF.Ln)

    # ---- negative (integral) part ----
    expN = wpool.tile([P, KC, D, D], bf16, tag="expN")
    nc.scalar.activation(expN[:], argN[:], AF.Exp)
    mulA = wpool.tile([P, KC, D, D], bf16, tag="mulA")
    redA = wpool.tile([P, KC, D], f32, tag="redA")
    for g in range(NG):
        gs = slice(4 * g, 4 * (g + 1))
        nc.vector.tensor_tensor(
            out=mulA[:, gs], in0=expN[:, gs],
            in1=aTflat.unsqueeze(1).broadcast_to([P, 4, D, D]), op=AL.mult)
        nc.vector.tensor_reduce(out=redA[:, gs], in_=mulA[:, gs], axis=AX.X,
                                op=AL.add)
    Y = wpool.tile([P, KC, D], f32, tag="Y")
    nc.vector.tensor_tensor(
        out=Y[:], in0=redA[:],
        in1=asumT.unsqueeze(1).broadcast_to([P, KC, D]), op=AL.subtract)
    nm = wpool.tile([P, KC, D], f32, tag="nm")
    nc.vector.tensor_tensor(out=nm[:], in0=onehot[:], in1=Y[:], op=AL.mult)
    negadj = wpool.tile([P, KC], f32, tag="negadj")
    nc.vector.tensor_reduce(out=negadj[:], in_=nm[:], axis=AX.X, op=AL.add)

    # ---- combine and reduce ----
    per_event = wpool.tile([P, KC], f32, tag="per_event")
    nc.vector.tensor_tensor(out=per_event[:], in0=loglam[:], in1=negadj[:],
                            op=AL.add)
    colsum = wpool.tile([P, 1], f32, tag="colsum")
    nc.vector.tensor_reduce(out=colsum[:], in_=per_event[:], axis=AX.X,
                            op=AL.add)
    totp = ps.tile([1, 1], f32, tag="totp")
    nc.tensor.matmul(totp[:], ones_col[:], colsum[:], start=True, stop=True)
    final = spool.tile([1, 1], f32, tag="final")
    nc.vector.tensor_tensor(out=final[:], in0=totp[:], in1=negconst,
                            op=AL.add)
    nc.sync.dma_start(out=out_ap, in_=final[:])


_CACHE = {}


def _build(Tval: float):
    key = float(Tval)
    if key in _CACHE:
        return _CACHE[key]
    nc = bacc.Bacc("TRN2", target_bir_lowering=False, debug=False)
    ins = {}
    for name, (shape, dt) in INPUTS.items():
        ins[name] = nc.dram_tensor(name, list(shape), dt,
                                   kind="ExternalInput").ap()
    out_ap = nc.dram_tensor("out", [1, 1], f32, kind="ExternalOutput").ap()
    with tile.TileContext(nc) as tc:
        with ExitStack() as ctx:
            _body(ctx, tc, ins, out_ap, Tval)
    nc.compile()
    _CACHE[key] = (nc, ins, out_ap)
    return _CACHE[key]


def host_prep(mu_raw, log_alpha, log_beta, Tval):
    """O(D^2) parameter transforms in float64 -> float32."""
    mu = np.log1p(np.exp(np.float64(mu_raw))).astype(np.float32)
    al = np.log1p(np.exp(np.float64(log_alpha))).astype(np.float32)
    be = np.log1p(np.exp(np.float64(log_beta))).astype(np.float32)
    ab = (al * be).astype(np.float32)

    pack_bf = np.zeros((P, 588), dtype=ml_dtypes.bfloat16)
    pack_bf[:, 0:128] = np.triu(np.ones((P, P), dtype=np.float32))
    pack_bf[:, 128:228] = np.broadcast_to(ab.reshape(-1), (P, RM))
    pack_bf[:, 228:328] = np.broadcast_to(al.T.reshape(-1), (P, RM))
    pack_bf[:, 328:428] = np.broadcast_to(be.T.reshape(-1), (P, RM))
    pack_bf[:, 428:588] = np.tile(np.arange(D, dtype=np.float32), KC)[None, :]

    pf_const = np.zeros((P, 285), dtype=np.float32)
    pf_const[:, 48:148] = np.broadcast_to(be.reshape(-1), (P, RM))
    pf_const[:, 148:158] = np.broadcast_to(mu - np.diag(ab), (P, D))
    pf_const[:, 158:168] = np.broadcast_to(al.sum(axis=0), (P, D))
    pf_const[:RM, 168:268] = np.eye(RM, dtype=np.float32)
    pf_const[0, 284] = np.float32(-Tval * mu.astype(np.float64).sum())
    return pack_bf, pf_const, be


def make_in_maps(time_points, event_types, mu_raw, log_alpha, log_beta, T):
    Tval = float(np.asarray(T))
    tp = np.asarray(time_points, dtype=np.float32)
    et = np.asarray(event_types).astype(np.float32)
    pack_bf, pf_const, be = host_prep(
        np.asarray(mu_raw), np.asarray(log_alpha), np.asarray(log_beta), Tval)
    in_maps = []
    for b in range(B):
        ts = tp[b, ::P]                       # [16] chunk reference times
        dtb = np.zeros(KC, dtype=np.float32)
        dtb[:-1] = ts[1:] - ts[:-1]
        pack_f32 = pf_const.copy()
        pack_f32[:, 0:16] = tp[b].reshape(KC, P).T
        pack_f32[:, 16:32] = et[b].reshape(KC, P).T
        pack_f32[:, 32:48] = ts[None, :]
        pack_f32[:RM, 268:284] = be.reshape(-1)[:, None] * dtb[None, :]
        in_maps.append({"pack_f32": pack_f32, "pack_bf": pack_bf})
    return in_maps, Tval


def kernel(time_points, event_types, mu_raw, log_alpha, log_beta, T):
    in_maps, Tval = make_in_maps(time_points, event_types, mu_raw,
                                 log_alpha, log_beta, T)
    nc, _, _ = _build(Tval)
    res = run_bass_kernel_spmd(nc, in_maps, list(range(B))).results
    out = np.array([res[b]["out"][0, 0] for b in range(B)], dtype=np.float32)
    return out


# revision 21
# speedup vs baseline: 1.0377x; 1.0377x over previous
"""Trainium2 Bass kernel for the exp-kernel multivariate Hawkes process
log-likelihood (B=8, N=2048, D=10).

Strategy
--------
Data-parallel over batch: core b computes batch row b fully on-chip and
returns one scalar; the host gathers the 8 scalars.

Per core the O(N^2) pairwise interaction is restructured into a chunked
O(N*D^2) algorithm (chunk size C=128 = partition count). Over (r,m) =
(receiver, trigger) type pairs (RM=100), with per-chunk reference times
ts_k:

  W[j,(r,m)]  = [e_j==m] * exp( b[r,m] (t_j - ts_k))
  P           = inclusive prefix of W over j within the chunk
                (PE matmul with upper-triangular ones)
  S_k[(r,m)]  = sum_{j < chunk k} exp(-b[r,m](ts_k - t_j))
                (inter-chunk state; affine scan over chunks)
  lam_i       = musub[e_i] + sum_{r,m} [e_i==r] exp(-b(t_i-ts)) ab[r,m] (P+S)[i,(r,m)]

The inclusive prefix counts the self pair j==i contributing exactly
ab[e_i,e_i]; host-precomputed musub = mu - diag(ab) cancels it.

The inter-chunk recurrence S_{k+1} = d_k*(S_k + Wsum_k) is ONE
`tensor_tensor_scan` in transposed layout [100,16]; per-chunk column
sums come from N=1 matmuls, and S is broadcast into PSUM with K=1
matmuls against a flattened S row.

The integral term uses the same masking trick with transposed tables:
  neg_ev_j = sum_m onehot[j,m] (asumT[m] - sum_d aT[m,d] exp(bT[m,d](t_j-T)))

Precision: exp arguments and all accumulations are fp32; post-exp
values, 0/1 masks, and matmul operands are bf16 (DVE 2x mode + PE
single-pass). Host-side work is limited to O(D^2) parameter softplus,
O(N) reshapes and the 16 chunk reference times.
"""
import numpy as np
from contextlib import ExitStack

import ml_dtypes
import concourse.bass as bass
import concourse.mybir as mybir
import concourse.tile as tile
from concourse import bacc
from concourse.bass_utils import run_bass_kernel_spmd

f32 = mybir.dt.float32
bf16 = mybir.dt.bfloat16
AL = mybir.AluOpType
AF = mybir.ActivationFunctionType
AX = mybir.AxisListType

P = 128          # partitions == chunk size
KC = 16          # number of chunks
D = 10           # event types
RM = D * D       # (receiver, trigger) pairs
N = P * KC       # 2048 events per batch row
B = 8            # batch == cores
NG = 4           # chunk groups (4 chunks per PSUM bank)

# packed DRAM inputs: name -> (shape, dtype)
INPUTS = {
    "pack_f32": ((P, 285), f32),    # t(16) e(16) tstart(16) bflat(100)
                                    # musub(10) asumT(10) ident(100)
                                    # bdtb(16) negconst(1)
    "pack_bf": ((P, 588), bf16),    # triu(128) abflat(100) aTflat(100)
                                    # bTflat(100) iota(160)
    "oht": ((D, N + 23), bf16),     # onehotT | [bT aT musub asum_hi asum_lo]
}


def _body(ctx: ExitStack, tc, ins, out_ap, Tval: float):
    nc = tc.nc
    cpool = ctx.enter_context(tc.tile_pool(name="cpool", bufs=1))
    wpool = ctx.enter_context(tc.tile_pool(name="wpool", bufs=1))
    spool = ctx.enter_context(tc.tile_pool(name="spool", bufs=1))
    pp = ctx.enter_context(tc.tile_pool(name="pp", bufs=1, space="PSUM"))
    ps = ctx.enter_context(tc.tile_pool(name="ps", bufs=1, space="PSUM"))

    # ---- load packed inputs on two parallel DMA queues ----
    pf = cpool.tile([P, 285], f32, tag="pf")
    nc.sync.dma_start(out=pf[:], in_=ins["pack_f32"])
    pb = cpool.tile([P, 588], bf16, tag="pb")
    nc.sync.dma_start(out=pb[:], in_=ins["pack_bf"])
    oht = cpool.tile([D, N + 23], bf16, tag="oht")
    nc.scalar.dma_start(out=oht[:], in_=ins["oht"])

    t128 = pf[:, 0:16]
    e128 = pf[:, 16:32]
    tstart = pf[:, 32:48]
    bflat = pf[:, 48:148].rearrange("p (r m) -> p r m", r=D)
    musub = pf[:, 148:158]
    asumT = pf[:, 158:168]
    ident = pf[0:RM, 168:268]
    bdtb = pf[0:RM, 268:284]
    negconst = pf[0:1, 284:285]
    triu = pb[:, 0:128]
    abflat = pb[:, 128:228].rearrange("p (r m) -> p r m", r=D)
    aTflat = pb[:, 228:328].rearrange("p (m d) -> p m d", m=D)
    bTflat = pb[:, 328:428].rearrange("p (m d) -> p m d", m=D)
    iota10 = pb[:, 428:588]

    # ---- constants ----
    ones_row = cpool.tile([1, P], bf16, tag="ones_row")
    nc.vector.memset(ones_row[:], 1.0)
    ones_col = cpool.tile([P, 1], f32, tag="ones_col")
    nc.vector.memset(ones_col[:], 1.0)
    ones_col_bf = cpool.tile([P, 1], bf16, tag="ones_col_bf")
    nc.vector.memset(ones_col_bf[:], 1.0)

    # ---- per-event scalars ----
    onehot = wpool.tile([P, KC, D], bf16, tag="onehot")
    nc.vector.tensor_tensor(
        out=onehot[:], in0=e128.unsqueeze(2).broadcast_to([P, KC, D]),
        in1=iota10.rearrange("p (k d) -> p k d", k=KC), op=AL.is_equal)
    trel = wpool.tile([P, KC], f32, tag="trel")
    nc.vector.tensor_tensor(out=trel[:], in0=t128, in1=tstart, op=AL.subtract)
    tau2 = wpool.tile([P, KC], f32, tag="tau2")
    nc.vector.tensor_scalar_add(tau2[:], t128, -Tval)

    # ---- positive-part exp pipeline (per group, so the PE starts early) ----
    argW = wpool.tile([P, KC, D, D], f32, tag="argW")
    expW = wpool.tile([P, KC, D, D], bf16, tag="expW")
    for g in range(NG):
        gs = slice(4 * g, 4 * (g + 1))
        nc.vector.tensor_tensor(
            out=argW[:, gs],
            in0=trel[:, gs].unsqueeze(2).unsqueeze(3)
                .broadcast_to([P, 4, D, D]),
            in1=bflat.unsqueeze(1).broadcast_to([P, 4, D, D]),
            op=AL.mult)
        nc.scalar.activation(expW[:, gs], argW[:, gs], AF.Exp)
    expU = wpool.tile([P, KC, D, D], bf16, tag="expU")
    nc.scalar.activation(expU[:], argW[:], AF.Exp, scale=-1.0)

    # W = expW * onehot[m]; all-bf16 SBUF => DVE 2x mode, per group for
    # PE overlap
    W = wpool.tile([P, KC, D, D], bf16, tag="W")
    for g in range(NG):
        gs = slice(4 * g, 4 * (g + 1))
        nc.vector.tensor_tensor(
            out=W[:, gs], in0=expW[:, gs],
            in1=onehot[:, gs].unsqueeze(2).broadcast_to([P, 4, D, D]),
            op=AL.mult)
    # expUab = exp(-argW) * ab;  U2ab = expUab * onehot[r]  (all-bf16, 2x)
    expUab = wpool.tile([P, KC, D, D], bf16, tag="expUab")
    nc.vector.tensor_tensor(
        out=expUab[:], in0=expU[:],
        in1=abflat.unsqueeze(1).broadcast_to([P, KC, D, D]), op=AL.mult)


    # ---- per-event gathered tables: grows[:,k,:] = onehotT_k^T @ tabs ----
    # cols: bT-row(10) | aT-row(10) | musub[e](1) | asumT[e](1)
    grows = ps.tile([P, KC, 23], f32, tag="grows")
    for k in range(KC):
        nc.tensor.matmul(grows[:, k], oht[:, k * P:(k + 1) * P],
                         oht[:, N:N + 23], start=True, stop=True)

    # ---- PE phase ----
    Pg = [pp.tile([P, 4, D, D], f32, tag=f"Pg{g}", name=f"Pg{g}")
          for g in range(NG)]
    wsumc = ps.tile([RM, KC], f32, tag="wsumc")
    for k in range(KC):
        nc.tensor.matmul(wsumc[:, k:k + 1],
                         W[:, k].rearrange("p r m -> p (r m)"),
                         ones_col_bf[:], start=True, stop=True)
    for g in range(NG):
        nc.tensor.matmul(Pg[g][:],
                         triu,
                         W[:, 4 * g:4 * (g + 1)].rearrange(
                             "p c r m -> p (c r m)"),
                         start=True, stop=False)

    # ---- inter-chunk affine scan (transposed layout [100,16]) ----
    decayT = spool.tile([RM, KC], f32, tag="decayT")
    nc.scalar.activation(decayT[:], bdtb, AF.Exp, scale=-1.0)
    V = spool.tile([RM, KC], f32, tag="V")
    nc.vector.tensor_tensor(out=V[:], in0=decayT[:], in1=wsumc[:], op=AL.mult)
    SCOL = spool.tile([RM, KC], f32, tag="SCOL")
    nc.vector.tensor_tensor_scan(SCOL[:], decayT[:], V[:], initial=0.0,
                                 op0=AL.mult, op1=AL.add)
    # SCOL[:, t] = S_{t+1}; transpose and flatten to a partition-0 row
    # (matmul operands must be quadrant-aligned), block 0 = S_0 = 0
    stp = ps.tile([KC, RM], f32, tag="stp")
    nc.tensor.transpose(stp[:], SCOL[:], ident)
    srows = spool.tile([KC, RM], bf16, tag="srows")
    nc.vector.tensor_copy(out=srows[:], in_=stp[:])
    sflat = spool.tile([1, KC * RM], bf16, tag="sflat")
    nc.vector.memset(sflat[:, :RM], 0.0)
    nc.sync.dma_start(out=sflat[:, RM:], in_=srows[:KC - 1, :])

    # batched S inject: one K=1 matmul per group broadcasts S_k to all rows
    for g in range(NG):
        nc.tensor.matmul(Pg[g][:], ones_row[:],
                         sflat[:, g * 4 * RM:(g + 1) * 4 * RM],
                         start=False, stop=True)

    # ---- positive part: lam via fused multiply-reduce per chunk ----
    # lam[:,k] = musub_ev[:,k] + sum_rm U2ab[:,k,rm] * (P+S)[:,k,rm]
    lamr = wpool.tile([P, KC], f32, tag="lamr")
    PM = wpool.tile([P, KC, D, D], bf16, tag="PM")
    G2 = wpool.tile([P, KC, D, D], bf16, tag="G2")
    for g in range(NG):
        gs = slice(4 * g, 4 * (g + 1))
        nc.vector.tensor_tensor(
            out=PM[:, gs], in0=Pg[g][:],
            in1=onehot[:, gs].unsqueeze(3).broadcast_to([P, 4, D, D]),
            op=AL.mult)
        nc.vector.tensor_tensor(out=G2[:, gs], in0=PM[:, gs],
                                in1=expUab[:, gs], op=AL.mult)
        nc.vector.tensor_reduce(
            out=lamr[:, gs],
            in_=G2[:, gs].rearrange("p c r m -> p c (r m)"),
            axis=AX.X, op=AL.add)
    lam = wpool.tile([P, KC], f32, tag="lam")
    nc.vector.tensor_tensor(out=lam[:], in0=lamr[:], in1=grows[:, :, 20],
                            op=AL.add)
    loglam = wpool.tile([P, KC], f32, tag="loglam")
    nc.scalar.activation(loglam[:], lam[:], AF.Ln)

    # ---- negative (integral) part via gathered per-event rows ----
    argN2 = wpool.tile([P, KC, D], f32, tag="argN2")
    nc.vector.tensor_tensor(
        out=argN2[:], in0=grows[:, :, 0:10],
        in1=tau2[:].unsqueeze(2).broadcast_to([P, KC, D]), op=AL.mult)
    expN2 = wpool.tile([P, KC, D], bf16, tag="expN2")
    nc.scalar.activation(expN2[:], argN2[:], AF.Exp)
    nmul = wpool.tile([P, KC, D], f32, tag="nmul")
    nc.vector.tensor_tensor(out=nmul[:], in0=expN2[:], in1=grows[:, :, 10:20],
                            op=AL.mult)
    negsub = wpool.tile([P, KC], f32, tag="negsub")
    nc.vector.tensor_reduce(out=negsub[:], in_=nmul[:], axis=AX.X, op=AL.add)

    # ---- combine and reduce ----
    pe1 = wpool.tile([P, KC], f32, tag="pe1")
    nc.vector.tensor_tensor(out=pe1[:], in0=loglam[:], in1=grows[:, :, 21],
                            op=AL.subtract)
    pe2 = wpool.tile([P, KC], f32, tag="pe2")
    nc.vector.tensor_tensor(out=pe2[:], in0=pe1[:], in1=grows[:, :, 22],
                            op=AL.subtract)
    per_event = wpool.tile([P, KC], f32, tag="per_event")
    nc.vector.tensor_tensor(out=per_event[:], in0=pe2[:], in1=negsub[:],
                            op=AL.add)
    colsum = wpool.tile([P, 1], f32, tag="colsum")
    nc.vector.tensor_reduce(out=colsum[:], in_=per_event[:], axis=AX.X,
                            op=AL.add)
    totp = ps.tile([1, 1], f32, tag="totp")
    nc.tensor.matmul(totp[:], ones_col[:], colsum[:], start=True, stop=True)
    final = spool.tile([1, 1], f32, tag="final")
    nc.vector.tensor_tensor(out=final[:], in0=totp[:], in1=negconst,
                            op=AL.add)
    nc.sync.dma_start(out=out_ap, in_=final[:])


_CACHE = {}


def _build(Tval: float):
    key = float(Tval)
    if key in _CACHE:
        return _CACHE[key]
    nc = bacc.Bacc("TRN2", target_bir_lowering=False, debug=False)
    ins = {}
    for name, (shape, dt) in INPUTS.items():
        ins[name] = nc.dram_tensor(name, list(shape), dt,
                                   kind="ExternalInput").ap()
    out_ap = nc.dram_tensor("out", [1, 1], f32, kind="ExternalOutput").ap()
    with tile.TileContext(nc) as tc:
        with ExitStack() as ctx:
            _body(ctx, tc, ins, out_ap, Tval)
    nc.compile()
    _CACHE[key] = (nc, ins, out_ap)
    return _CACHE[key]


def host_prep(mu_raw, log_alpha, log_beta, Tval):
    """O(D^2) parameter transforms in float64 -> float32."""
    mu = np.log1p(np.exp(np.float64(mu_raw))).astype(np.float32)
    al = np.log1p(np.exp(np.float64(log_alpha))).astype(np.float32)
    be = np.log1p(np.exp(np.float64(log_beta))).astype(np.float32)
    ab = (al * be).astype(np.float32)

    pack_bf = np.zeros((P, 588), dtype=ml_dtypes.bfloat16)
    pack_bf[:, 0:128] = np.triu(np.ones((P, P), dtype=np.float32))
    pack_bf[:, 128:228] = np.broadcast_to(ab.reshape(-1), (P, RM))
    pack_bf[:, 228:328] = np.broadcast_to(al.T.reshape(-1), (P, RM))
    pack_bf[:, 328:428] = np.broadcast_to(be.T.reshape(-1), (P, RM))
    pack_bf[:, 428:588] = np.tile(np.arange(D, dtype=np.float32), KC)[None, :]

    pf_const = np.zeros((P, 285), dtype=np.float32)
    pf_const[:, 48:148] = np.broadcast_to(be.reshape(-1), (P, RM))
    pf_const[:RM, 168:268] = np.eye(RM, dtype=np.float32)
    pf_const[0, 284] = np.float32(-Tval * mu.astype(np.float64).sum())

    tabs = np.zeros((D, 23), dtype=np.float32)
    tabs[:, 0:10] = be.T
    tabs[:, 10:20] = al.T
    tabs[:, 20] = mu - np.diag(ab)
    asum = al.sum(axis=0)
    asum_hi = asum.astype(ml_dtypes.bfloat16).astype(np.float32)
    tabs[:, 21] = asum_hi
    tabs[:, 22] = asum - asum_hi
    return pack_bf, pf_const, be, tabs


def make_in_maps(time_points, event_types, mu_raw, log_alpha, log_beta, T):
    Tval = float(np.asarray(T))
    tp = np.asarray(time_points, dtype=np.float32)
    et = np.asarray(event_types).astype(np.float32)
    pack_bf, pf_const, be, tabs = host_prep(
        np.asarray(mu_raw), np.asarray(log_alpha), np.asarray(log_beta), Tval)
    in_maps = []
    for b in range(B):
        ts = tp[b, ::P]                       # [16] chunk reference times
        dtb = np.zeros(KC, dtype=np.float32)
        dtb[:-1] = ts[1:] - ts[:-1]
        pack_f32 = pf_const.copy()
        pack_f32[:, 0:16] = tp[b].reshape(KC, P).T
        pack_f32[:, 16:32] = et[b].reshape(KC, P).T
        pack_f32[:, 32:48] = ts[None, :]
        pack_f32[:RM, 268:284] = be.reshape(-1)[:, None] * dtb[None, :]
        oht = np.zeros((D, N + 23), dtype=ml_dtypes.bfloat16)
        oht[:, 0:N] = (et[b][None, :] == np.arange(D, dtype=np.float32)[:, None])
        oht[:, N:N + 23] = tabs
        in_maps.append({"pack_f32": pack_f32, "pack_bf": pack_bf, "oht": oht})
    return in_maps, Tval


def kernel(time_points, event_types, mu_raw, log_alpha, log_beta, T):
    in_maps, Tval = make_in_maps(time_points, event_types, mu_raw,
                                 log_alpha, log_beta, T)
    nc, _, _ = _build(Tval)
    res = run_bass_kernel_spmd(nc, in_maps, list(range(B))).results
    out = np.array([res[b]["out"][0, 0] for b in range(B)], dtype=np.float32)
    return out


# revision 23
# speedup vs baseline: 1.1275x; 1.0865x over previous
"""Trainium2 Bass kernel for the exp-kernel multivariate Hawkes process
log-likelihood (B=8, N=2048, D=10).

Strategy
--------
Data-parallel over batch: core b computes batch row b fully on-chip and
returns one scalar; the host gathers the 8 scalars.

Per core the O(N^2) pairwise interaction is restructured into a chunked
O(N*D^2) algorithm (chunk size C=128 = partition count). Over (r,m) =
(receiver, trigger) type pairs (RM=100), with per-chunk reference times
ts_k:

  W[j,(r,m)]  = [e_j==m] * exp( b[r,m] (t_j - ts_k))
  P           = inclusive prefix of W over j within the chunk
                (PE matmul with upper-triangular ones)
  S_k[(r,m)]  = sum_{j < chunk k} exp(-b[r,m](ts_k - t_j))
                (inter-chunk state; affine scan over chunks)
  lam_i       = musub[e_i] + sum_{r,m} [e_i==r] exp(-b(t_i-ts)) ab[r,m] (P+S)[i,(r,m)]

The inclusive prefix counts the self pair j==i contributing exactly
ab[e_i,e_i]; host-precomputed musub = mu - diag(ab) cancels it.

The inter-chunk recurrence S_{k+1} = d_k*(S_k + Wsum_k) is ONE
`tensor_tensor_scan` in transposed layout [100,16]; per-chunk column
sums come from N=1 matmuls, and S is broadcast into PSUM with K=1
matmuls against a flattened S row.

The integral term uses the same masking trick with transposed tables:
  neg_ev_j = sum_m onehot[j,m] (asumT[m] - sum_d aT[m,d] exp(bT[m,d](t_j-T)))

Precision: exp arguments and all accumulations are fp32; post-exp
values, 0/1 masks, and matmul operands are bf16 (DVE 2x mode + PE
single-pass). Host-side work is limited to O(D^2) parameter softplus,
O(N) reshapes and the 16 chunk reference times.
"""
import numpy as np
from contextlib import ExitStack

import ml_dtypes
import concourse.bass as bass
import concourse.mybir as mybir
import concourse.tile as tile
from concourse import bacc
from concourse.bass_utils import run_bass_kernel_spmd

f32 = mybir.dt.float32
bf16 = mybir.dt.bfloat16
AL = mybir.AluOpType
AF = mybir.ActivationFunctionType
AX = mybir.AxisListType

P = 128          # partitions == chunk size
KC = 16          # number of chunks
D = 10           # event types
RM = D * D       # (receiver, trigger) pairs
N = P * KC       # 2048 events per batch row
B = 8            # batch == cores
NG = 4           # chunk groups (4 chunks per PSUM bank)

# packed DRAM inputs: name -> (shape, dtype)
INPUTS = {
    "pack_f32": ((P, 285), f32),    # t(16) e(16) tstart(16) bflat(100)
                                    # musub(10) asumT(10) ident(100)
                                    # bdtb(16) negconst(1)
    "pack_bf": ((P, 588), bf16),    # triu(128) abflat(100) aTflat(100)
                                    # bTflat(100) iota(160)
    "oht": ((D, N + 23), bf16),     # onehotT | [bT aT musub asum_hi asum_lo]
}


def _body(ctx: ExitStack, tc, ins, out_ap, Tval: float):
    nc = tc.nc
    cpool = ctx.enter_context(tc.tile_pool(name="cpool", bufs=1))
    wpool = ctx.enter_context(tc.tile_pool(name="wpool", bufs=1))
    spool = ctx.enter_context(tc.tile_pool(name="spool", bufs=1))
    pp = ctx.enter_context(tc.tile_pool(name="pp", bufs=1, space="PSUM"))
    ps = ctx.enter_context(tc.tile_pool(name="ps", bufs=1, space="PSUM"))

    # ---- load packed inputs on two parallel DMA queues ----
    pf = cpool.tile([P, 285], f32, tag="pf")
    nc.sync.dma_start(out=pf[:], in_=ins["pack_f32"])
    pb = cpool.tile([P, 588], bf16, tag="pb")
    nc.sync.dma_start(out=pb[:], in_=ins["pack_bf"])
    oht = cpool.tile([D, N + 23], bf16, tag="oht")
    nc.scalar.dma_start(out=oht[:], in_=ins["oht"])

    t128 = pf[:, 0:16]
    e128 = pf[:, 16:32]
    tstart = pf[:, 32:48]
    bflat = pf[:, 48:148].rearrange("p (r m) -> p r m", r=D)
    musub = pf[:, 148:158]
    asumT = pf[:, 158:168]
    ident = pf[0:RM, 168:268]
    bdtb = pf[0:RM, 268:284]
    negconst = pf[0:1, 284:285]
    triu = pb[:, 0:128]
    abflat = pb[:, 128:228].rearrange("p (r m) -> p r m", r=D)
    aTflat = pb[:, 228:328].rearrange("p (m d) -> p m d", m=D)
    bTflat = pb[:, 328:428].rearrange("p (m d) -> p m d", m=D)
    iota10 = pb[:, 428:588]

    # ---- constants ----
    ones_row = cpool.tile([1, P], bf16, tag="ones_row")
    nc.vector.memset(ones_row[:], 1.0)
    ones_col = cpool.tile([P, 1], f32, tag="ones_col")
    nc.vector.memset(ones_col[:], 1.0)
    ones_col_bf = cpool.tile([P, 1], bf16, tag="ones_col_bf")
    nc.vector.memset(ones_col_bf[:], 1.0)

    # ---- per-event scalars ----
    onehot = wpool.tile([P, KC, D], bf16, tag="onehot")
    nc.vector.tensor_tensor(
        out=onehot[:], in0=e128.unsqueeze(2).broadcast_to([P, KC, D]),
        in1=iota10.rearrange("p (k d) -> p k d", k=KC), op=AL.is_equal)
    trel = wpool.tile([P, KC], f32, tag="trel")
    nc.vector.tensor_tensor(out=trel[:], in0=t128, in1=tstart, op=AL.subtract)
    tau2 = wpool.tile([P, KC], f32, tag="tau2")
    nc.vector.tensor_scalar_add(tau2[:], t128, -Tval)

    # ---- positive-part exp pipeline (per group, so the PE starts early) ----
    argW = wpool.tile([P, KC, D, D], f32, tag="argW")
    expW = wpool.tile([P, KC, D, D], bf16, tag="expW")
    for g in range(NG):
        gs = slice(4 * g, 4 * (g + 1))
        nc.vector.tensor_tensor(
            out=argW[:, gs],
            in0=trel[:, gs].unsqueeze(2).unsqueeze(3)
                .broadcast_to([P, 4, D, D]),
            in1=bflat.unsqueeze(1).broadcast_to([P, 4, D, D]),
            op=AL.mult)
        nc.scalar.activation(expW[:, gs], argW[:, gs], AF.Exp)
    expU = wpool.tile([P, KC, D, D], bf16, tag="expU")
    nc.scalar.activation(expU[:], argW[:], AF.Exp, scale=-1.0)

    # W = expW * onehot[m]; all-bf16 SBUF => DVE 2x mode, per group for
    # PE overlap
    W = wpool.tile([P, KC, D, D], bf16, tag="W")
    for g in range(NG):
        gs = slice(4 * g, 4 * (g + 1))
        nc.vector.tensor_tensor(
            out=W[:, gs], in0=expW[:, gs],
            in1=onehot[:, gs].unsqueeze(2).broadcast_to([P, 4, D, D]),
            op=AL.mult)
    # expUab = exp(-argW) * ab;  U2ab = expUab * onehot[r]  (all-bf16, 2x)
    expUab = wpool.tile([P, KC, D, D], bf16, tag="expUab")
    nc.vector.tensor_tensor(
        out=expUab[:], in0=expU[:],
        in1=abflat.unsqueeze(1).broadcast_to([P, KC, D, D]), op=AL.mult)


    # ---- per-event gathered tables: grows[:,k,:] = onehotT_k^T @ tabs ----
    # cols: bT-row(10) | aT-row(10) | musub[e](1) | asumT[e](1)
    grows = ps.tile([P, KC, 23], f32, tag="grows")
    for k in range(KC):
        nc.tensor.matmul(grows[:, k], oht[:, k * P:(k + 1) * P],
                         oht[:, N:N + 23], start=True, stop=True)

    # ---- negative (integral) part via gathered per-event rows ----
    argN2 = wpool.tile([P, KC, D], f32, tag="argN2")
    nc.vector.tensor_tensor(
        out=argN2[:], in0=grows[:, :, 0:10],
        in1=tau2[:].unsqueeze(2).broadcast_to([P, KC, D]), op=AL.mult)
    expN2 = wpool.tile([P, KC, D], bf16, tag="expN2")
    nc.scalar.activation(expN2[:], argN2[:], AF.Exp)
    nmul = wpool.tile([P, KC, D], f32, tag="nmul")
    nc.vector.tensor_tensor(out=nmul[:], in0=expN2[:], in1=grows[:, :, 10:20],
                            op=AL.mult)
    negsub = wpool.tile([P, KC], f32, tag="negsub")
    nc.vector.tensor_reduce(out=negsub[:], in_=nmul[:], axis=AX.X, op=AL.add)



    # ---- PE phase ----
    Pg = [pp.tile([P, 4, D, D], f32, tag=f"Pg{g}", name=f"Pg{g}")
          for g in range(NG)]
    wsumc = ps.tile([RM, KC], f32, tag="wsumc")
    for k in range(KC):
        nc.tensor.matmul(wsumc[:, k:k + 1],
                         W[:, k].rearrange("p r m -> p (r m)"),
                         ones_col_bf[:], start=True, stop=True)

    # ---- inter-chunk affine scan (transposed layout [100,16]) ----
    decayT = spool.tile([RM, KC], f32, tag="decayT")
    nc.scalar.activation(decayT[:], bdtb, AF.Exp, scale=-1.0)
    V = spool.tile([RM, KC], f32, tag="V")
    nc.vector.tensor_tensor(out=V[:], in0=decayT[:], in1=wsumc[:], op=AL.mult)
    SCOL = spool.tile([RM, KC], f32, tag="SCOL")
    nc.vector.tensor_tensor_scan(SCOL[:], decayT[:], V[:], initial=0.0,
                                 op0=AL.mult, op1=AL.add)
    # SCOL[:, t] = S_{t+1}; transpose and flatten to a partition-0 row
    # (matmul operands must be quadrant-aligned), block 0 = S_0 = 0
    stp = ps.tile([KC, RM], f32, tag="stp")
    nc.tensor.transpose(stp[:], SCOL[:], ident)
    srows = spool.tile([KC, RM], bf16, tag="srows")
    nc.vector.tensor_copy(out=srows[:], in_=stp[:])
    sflat = spool.tile([1, KC * RM], bf16, tag="sflat")
    nc.vector.memset(sflat[:, :RM], 0.0)
    nc.sync.dma_start(out=sflat[:, RM:], in_=srows[:KC - 1, :])

    # batched inclusive prefix (fills the PE while the S row is being built)
    for g in range(NG):
        nc.tensor.matmul(Pg[g][:],
                         triu,
                         W[:, 4 * g:4 * (g + 1)].rearrange(
                             "p c r m -> p (c r m)"),
                         start=True, stop=False)

    # batched S inject: one K=1 matmul per group broadcasts S_k to all rows
    for g in range(NG):
        nc.tensor.matmul(Pg[g][:], ones_row[:],
                         sflat[:, g * 4 * RM:(g + 1) * 4 * RM],
                         start=False, stop=True)

    # ---- positive part: lam via fused multiply-reduce per chunk ----
    # lam[:,k] = musub_ev[:,k] + sum_rm U2ab[:,k,rm] * (P+S)[:,k,rm]
    lamr = wpool.tile([P, KC], f32, tag="lamr")
    PM = wpool.tile([P, KC, D, D], bf16, tag="PM")
    G2 = wpool.tile([P, KC, D, D], bf16, tag="G2")
    for g in range(NG):
        gs = slice(4 * g, 4 * (g + 1))
        nc.vector.tensor_tensor(
            out=PM[:, gs], in0=Pg[g][:],
            in1=onehot[:, gs].unsqueeze(3).broadcast_to([P, 4, D, D]),
            op=AL.mult)
        nc.vector.tensor_tensor(out=G2[:, gs], in0=PM[:, gs],
                                in1=expUab[:, gs], op=AL.mult)
        nc.vector.tensor_reduce(
            out=lamr[:, gs],
            in_=G2[:, gs].rearrange("p c r m -> p c (r m)"),
            axis=AX.X, op=AL.add)
    lam = wpool.tile([P, KC], f32, tag="lam")
    nc.vector.tensor_tensor(out=lam[:], in0=lamr[:], in1=grows[:, :, 20],
                            op=AL.add)
    loglam = wpool.tile([P, KC], f32, tag="loglam")
    nc.scalar.activation(loglam[:], lam[:], AF.Ln)

    # ---- combine and reduce ----
    pe1 = wpool.tile([P, KC], f32, tag="pe1")
    nc.vector.tensor_tensor(out=pe1[:], in0=loglam[:], in1=grows[:, :, 21],
                            op=AL.subtract)
    pe2 = wpool.tile([P, KC], f32, tag="pe2")
    nc.vector.tensor_tensor(out=pe2[:], in0=pe1[:], in1=grows[:, :, 22],
                            op=AL.subtract)
    per_event = wpool.tile([P, KC], f32, tag="per_event")
    nc.vector.tensor_tensor(out=per_event[:], in0=pe2[:], in1=negsub[:],
                            op=AL.add)
    colsum = wpool.tile([P, 1], f32, tag="colsum")
    nc.vector.tensor_reduce(out=colsum[:], in_=per_event[:], axis=AX.X,
                            op=AL.add)
    totp = ps.tile([1, 1], f32, tag="totp")
    nc.tensor.matmul(totp[:], ones_col[:], colsum[:], start=True, stop=True)
    final = spool.tile([1, 1], f32, tag="final")
    nc.vector.tensor_tensor(out=final[:], in0=totp[:], in1=negconst,
                            op=AL.add)
    nc.sync.dma_start(out=out_ap, in_=final[:])


_CACHE = {}


def _build(Tval: float):
    key = float(Tval)
    if key in _CACHE:
        return _CACHE[key]
    nc = bacc.Bacc("TRN2", target_bir_lowering=False, debug=False)
    ins = {}
    for name, (shape, dt) in INPUTS.items():
        ins[name] = nc.dram_tensor(name, list(shape), dt,
                                   kind="ExternalInput").ap()
    out_ap = nc.dram_tensor("out", [1, 1], f32, kind="ExternalOutput").ap()
    with tile.TileContext(nc) as tc:
        with ExitStack() as ctx:
            _body(ctx, tc, ins, out_ap, Tval)
    nc.compile()
    _CACHE[key] = (nc, ins, out_ap)
    return _CACHE[key]


def host_prep(mu_raw, log_alpha, log_beta, Tval):
    """O(D^2) parameter transforms in float64 -> float32."""
    mu = np.log1p(np.exp(np.float64(mu_raw))).astype(np.float32)
    al = np.log1p(np.exp(np.float64(log_alpha))).astype(np.float32)
    be = np.log1p(np.exp(np.float64(log_beta))).astype(np.float32)
    ab = (al * be).astype(np.float32)

    pack_bf = np.zeros((P, 588), dtype=ml_dtypes.bfloat16)
    pack_bf[:, 0:128] = np.triu(np.ones((P, P), dtype=np.float32))
    pack_bf[:, 128:228] = np.broadcast_to(ab.reshape(-1), (P, RM))
    pack_bf[:, 228:328] = np.broadcast_to(al.T.reshape(-1), (P, RM))
    pack_bf[:, 328:428] = np.broadcast_to(be.T.reshape(-1), (P, RM))
    pack_bf[:, 428:588] = np.tile(np.arange(D, dtype=np.float32), KC)[None, :]

    pf_const = np.zeros((P, 285), dtype=np.float32)
    pf_const[:, 48:148] = np.broadcast_to(be.reshape(-1), (P, RM))
    pf_const[:RM, 168:268] = np.eye(RM, dtype=np.float32)
    pf_const[0, 284] = np.float32(-Tval * mu.astype(np.float64).sum())

    tabs = np.zeros((D, 23), dtype=np.float32)
    tabs[:, 0:10] = be.T
    tabs[:, 10:20] = al.T
    tabs[:, 20] = mu - np.diag(ab)
    asum = al.sum(axis=0)
    asum_hi = asum.astype(ml_dtypes.bfloat16).astype(np.float32)
    tabs[:, 21] = asum_hi
    tabs[:, 22] = asum - asum_hi
    return pack_bf, pf_const, be, tabs


def make_in_maps(time_points, event_types, mu_raw, log_alpha, log_beta, T):
    Tval = float(np.asarray(T))
    tp = np.asarray(time_points, dtype=np.float32)
    et = np.asarray(event_types).astype(np.float32)
    pack_bf, pf_const, be, tabs = host_prep(
        np.asarray(mu_raw), np.asarray(log_alpha), np.asarray(log_beta), Tval)
    in_maps = []
    for b in range(B):
        ts = tp[b, ::P]                       # [16] chunk reference times
        dtb = np.zeros(KC, dtype=np.float32)
        dtb[:-1] = ts[1:] - ts[:-1]
        pack_f32 = pf_const.copy()
        pack_f32[:, 0:16] = tp[b].reshape(KC, P).T
        pack_f32[:, 16:32] = et[b].reshape(KC, P).T
        pack_f32[:, 32:48] = ts[None, :]
        pack_f32[:RM, 268:284] = be.reshape(-1)[:, None] * dtb[None, :]
        oht = np.zeros((D, N + 23), dtype=ml_dtypes.bfloat16)
        oht[:, 0:N] = (et[b][None, :] == np.arange(D, dtype=np.float32)[:, None])
        oht[:, N:N + 23] = tabs
        in_maps.append({"pack_f32": pack_f32, "pack_bf": pack_bf, "oht": oht})
    return in_maps, Tval


def kernel(time_points, event_types, mu_raw, log_alpha, log_beta, T):
    in_maps, Tval = make_in_maps(time_points, event_types, mu_raw,
                                 log_alpha, log_beta, T)
    nc, _, _ = _build(Tval)
    res = run_bass_kernel_spmd(nc, in_maps, list(range(B))).results
    out = np.array([res[b]["out"][0, 0] for b in range(B)], dtype=np.float32)
    return out


# revision 24
# speedup vs baseline: 1.1444x; 1.0150x over previous
"""Trainium2 Bass kernel for the exp-kernel multivariate Hawkes process
log-likelihood (B=8, N=2048, D=10).

Strategy
--------
Data-parallel over batch: core b computes batch row b fully on-chip and
returns one scalar; the host gathers the 8 scalars.

Per core the O(N^2) pairwise interaction is restructured into a chunked
O(N*D^2) algorithm (chunk size C=128 = partition count). Over (r,m) =
(receiver, trigger) type pairs (RM=100), with per-chunk reference times
ts_k:

  W[j,(r,m)]  = [e_j==m] * exp( b[r,m] (t_j - ts_k))
  P           = inclusive prefix of W over j within the chunk
                (PE matmul with upper-triangular ones)
  S_k[(r,m)]  = sum_{j < chunk k} exp(-b[r,m](ts_k - t_j))
                (inter-chunk state; affine scan over chunks)
  lam_i       = musub[e_i] + sum_{r,m} [e_i==r] exp(-b(t_i-ts)) ab[r,m] (P+S)[i,(r,m)]

The inclusive prefix counts the self pair j==i contributing exactly
ab[e_i,e_i]; host-precomputed musub = mu - diag(ab) cancels it.

The inter-chunk recurrence S_{k+1} = d_k*(S_k + Wsum_k) is ONE
`tensor_tensor_scan` in transposed layout [100,16]; per-chunk column
sums come from N=1 matmuls, and S is broadcast into PSUM with K=1
matmuls against a flattened S row.

The integral term uses the same masking trick with transposed tables:
  neg_ev_j = sum_m onehot[j,m] (asumT[m] - sum_d aT[m,d] exp(bT[m,d](t_j-T)))

Precision: exp arguments and all accumulations are fp32; post-exp
values, 0/1 masks, and matmul operands are bf16 (DVE 2x mode + PE
single-pass). Host-side work is limited to O(D^2) parameter softplus,
O(N) reshapes and the 16 chunk reference times.
"""
import numpy as np
from contextlib import ExitStack

import ml_dtypes
import concourse.bass as bass
import concourse.mybir as mybir
import concourse.tile as tile
from concourse import bacc
from concourse.bass_utils import run_bass_kernel_spmd

f32 = mybir.dt.float32
bf16 = mybir.dt.bfloat16
AL = mybir.AluOpType
AF = mybir.ActivationFunctionType
AX = mybir.AxisListType

P = 128          # partitions == chunk size
KC = 16          # number of chunks
D = 10           # event types
RM = D * D       # (receiver, trigger) pairs
N = P * KC       # 2048 events per batch row
B = 8            # batch == cores
NG = 4           # chunk groups (4 chunks per PSUM bank)

# packed DRAM inputs: name -> (shape, dtype)
INPUTS = {
    "pack_f32": ((P, 285), f32),    # t(16) e(16) tstart(16) bflat(100)
                                    # musub(10) asumT(10) ident(100)
                                    # bdtb(16) negconst(1)
    "pack_bf": ((P, 588), bf16),    # triu(128) abflat(100) aTflat(100)
                                    # bTflat(100) iota(160)
    "oht": ((D, N + 23), bf16),     # onehotT | [bT aT musub asum_hi asum_lo]
}


def _body(ctx: ExitStack, tc, ins, out_ap, Tval: float):
    nc = tc.nc
    cpool = ctx.enter_context(tc.tile_pool(name="cpool", bufs=1))
    wpool = ctx.enter_context(tc.tile_pool(name="wpool", bufs=1))
    spool = ctx.enter_context(tc.tile_pool(name="spool", bufs=1))
    pp = ctx.enter_context(tc.tile_pool(name="pp", bufs=1, space="PSUM"))
    ps = ctx.enter_context(tc.tile_pool(name="ps", bufs=1, space="PSUM"))

    # ---- load packed inputs on two parallel DMA queues ----
    pf = cpool.tile([P, 285], f32, tag="pf")
    nc.sync.dma_start(out=pf[:], in_=ins["pack_f32"])
    pb = cpool.tile([P, 588], bf16, tag="pb")
    nc.sync.dma_start(out=pb[:], in_=ins["pack_bf"])
    oht = cpool.tile([D, N + 23], bf16, tag="oht")
    nc.scalar.dma_start(out=oht[:], in_=ins["oht"])

    t128 = pf[:, 0:16]
    e128 = pf[:, 16:32]
    tstart = pf[:, 32:48]
    bflat = pf[:, 48:148].rearrange("p (r m) -> p r m", r=D)
    musub = pf[:, 148:158]
    asumT = pf[:, 158:168]
    ident = pf[0:RM, 168:268]
    bdtb = pf[0:RM, 268:284]
    negconst = pf[0:1, 284:285]
    triu = pb[:, 0:128]
    abflat = pb[:, 128:228].rearrange("p (r m) -> p r m", r=D)
    aTflat = pb[:, 228:328].rearrange("p (m d) -> p m d", m=D)
    bTflat = pb[:, 328:428].rearrange("p (m d) -> p m d", m=D)
    iota10 = pb[:, 428:588]

    # ---- constants ----
    ones_row = cpool.tile([1, P], bf16, tag="ones_row")
    nc.vector.memset(ones_row[:], 1.0)
    ones_col = cpool.tile([P, 1], f32, tag="ones_col")
    nc.vector.memset(ones_col[:], 1.0)
    ones_col_bf = cpool.tile([P, 1], bf16, tag="ones_col_bf")
    nc.vector.memset(ones_col_bf[:], 1.0)

    # ---- per-event scalars ----
    onehot = wpool.tile([P, KC, D], bf16, tag="onehot")
    nc.vector.tensor_tensor(
        out=onehot[:], in0=e128.unsqueeze(2).broadcast_to([P, KC, D]),
        in1=iota10.rearrange("p (k d) -> p k d", k=KC), op=AL.is_equal)
    trel = wpool.tile([P, KC], f32, tag="trel")
    nc.vector.tensor_tensor(out=trel[:], in0=t128, in1=tstart, op=AL.subtract)
    tau2 = wpool.tile([P, KC], f32, tag="tau2")
    nc.vector.tensor_scalar_add(tau2[:], t128, -Tval)

    # ---- positive-part exp pipeline (per group, so the PE starts early) ----
    argW = wpool.tile([P, KC, D, D], f32, tag="argW")
    expW = wpool.tile([P, KC, D, D], bf16, tag="expW")
    for g in range(NG):
        gs = slice(4 * g, 4 * (g + 1))
        nc.vector.tensor_tensor(
            out=argW[:, gs],
            in0=trel[:, gs].unsqueeze(2).unsqueeze(3)
                .broadcast_to([P, 4, D, D]),
            in1=bflat.unsqueeze(1).broadcast_to([P, 4, D, D]),
            op=AL.mult)
        nc.scalar.activation(expW[:, gs], argW[:, gs], AF.Exp)
    expU = wpool.tile([P, KC, D, D], bf16, tag="expU")
    nc.scalar.activation(expU[:], argW[:], AF.Exp, scale=-1.0)

    # W = expW * onehot[m]; all-bf16 SBUF => DVE 2x mode, per group for
    # PE overlap
    W = wpool.tile([P, KC, D, D], bf16, tag="W")
    for g in range(NG):
        gs = slice(4 * g, 4 * (g + 1))
        nc.vector.tensor_tensor(
            out=W[:, gs], in0=expW[:, gs],
            in1=onehot[:, gs].unsqueeze(2).broadcast_to([P, 4, D, D]),
            op=AL.mult)
    # expUab = exp(-argW) * ab;  U2ab = expUab * onehot[r]  (all-bf16, 2x)
    expUab = wpool.tile([P, KC, D, D], bf16, tag="expUab")
    nc.vector.tensor_tensor(
        out=expUab[:], in0=expU[:],
        in1=abflat.unsqueeze(1).broadcast_to([P, KC, D, D]), op=AL.mult)


    # ---- per-event gathered tables: grows[:,k,:] = onehotT_k^T @ tabs ----
    # cols: bT-row(10) | aT-row(10) | musub[e](1) | asumT[e](1)
    grows = ps.tile([P, KC, 23], f32, tag="grows")
    for k in range(KC):
        nc.tensor.matmul(grows[:, k], oht[:, k * P:(k + 1) * P],
                         oht[:, N:N + 23], start=True, stop=True)

    # ---- negative (integral) part via gathered per-event rows ----
    argN2 = wpool.tile([P, KC, D], f32, tag="argN2")
    nc.vector.tensor_tensor(
        out=argN2[:], in0=grows[:, :, 0:10],
        in1=tau2[:].unsqueeze(2).broadcast_to([P, KC, D]), op=AL.mult)
    expN2 = wpool.tile([P, KC, D], bf16, tag="expN2")
    nc.scalar.activation(expN2[:], argN2[:], AF.Exp)
    nmul = wpool.tile([P, KC, D], f32, tag="nmul")
    nc.vector.tensor_tensor(out=nmul[:], in0=expN2[:], in1=grows[:, :, 10:20],
                            op=AL.mult)
    negsub = wpool.tile([P, KC], f32, tag="negsub")
    nc.vector.tensor_reduce(out=negsub[:], in_=nmul[:], axis=AX.X, op=AL.add)
    ngt1 = wpool.tile([P, KC], f32, tag="ngt1")
    nc.vector.tensor_tensor(out=ngt1[:], in0=negsub[:], in1=grows[:, :, 21],
                            op=AL.subtract)
    negtot = wpool.tile([P, KC], f32, tag="negtot")
    nc.vector.tensor_tensor(out=negtot[:], in0=ngt1[:], in1=grows[:, :, 22],
                            op=AL.subtract)



    # ---- PE phase ----
    Pg = [pp.tile([P, 4, D, D], f32, tag=f"Pg{g}", name=f"Pg{g}")
          for g in range(NG)]
    wsumc = ps.tile([RM, KC], f32, tag="wsumc")
    for k in range(KC):
        nc.tensor.matmul(wsumc[:, k:k + 1],
                         W[:, k].rearrange("p r m -> p (r m)"),
                         ones_col_bf[:], start=True, stop=True)

    # ---- inter-chunk affine scan (transposed layout [100,16]) ----
    decayT = spool.tile([RM, KC], f32, tag="decayT")
    nc.scalar.activation(decayT[:], bdtb, AF.Exp, scale=-1.0)
    V = spool.tile([RM, KC], f32, tag="V")
    nc.vector.tensor_tensor(out=V[:], in0=decayT[:], in1=wsumc[:], op=AL.mult)
    SCOL = spool.tile([RM, KC], f32, tag="SCOL")
    nc.vector.tensor_tensor_scan(SCOL[:], decayT[:], V[:], initial=0.0,
                                 op0=AL.mult, op1=AL.add)
    # SCOL[:, t] = S_{t+1}; transpose and flatten to a partition-0 row
    # (matmul operands must be quadrant-aligned), block 0 = S_0 = 0
    stp = ps.tile([KC, RM], f32, tag="stp")
    nc.tensor.transpose(stp[:], SCOL[:], ident)
    srows = spool.tile([KC, RM], bf16, tag="srows")
    nc.vector.tensor_copy(out=srows[:], in_=stp[:])
    sflat = spool.tile([1, KC * RM], bf16, tag="sflat")
    nc.vector.memset(sflat[:, :RM], 0.0)
    nc.scalar.dma_start(out=sflat[:, RM:], in_=srows[:KC - 1, :])

    # batched inclusive prefix (fills the PE while the S row is being built)
    for g in range(NG):
        nc.tensor.matmul(Pg[g][:],
                         triu,
                         W[:, 4 * g:4 * (g + 1)].rearrange(
                             "p c r m -> p (c r m)"),
                         start=True, stop=False)

    # batched S inject: one K=1 matmul per group broadcasts S_k to all rows
    for g in range(NG):
        nc.tensor.matmul(Pg[g][:], ones_row[:],
                         sflat[:, g * 4 * RM:(g + 1) * 4 * RM],
                         start=False, stop=True)

    # ---- positive part: lam via fused multiply-reduce per chunk ----
    # lam[:,k] = musub_ev[:,k] + sum_rm U2ab[:,k,rm] * (P+S)[:,k,rm]
    lamr = wpool.tile([P, KC], f32, tag="lamr")
    PM = wpool.tile([P, KC, D, D], bf16, tag="PM")
    G2 = wpool.tile([P, KC, D, D], bf16, tag="G2")
    for g in range(NG):
        gs = slice(4 * g, 4 * (g + 1))
        nc.vector.tensor_tensor(
            out=PM[:, gs], in0=Pg[g][:],
            in1=onehot[:, gs].unsqueeze(3).broadcast_to([P, 4, D, D]),
            op=AL.mult)
        nc.vector.tensor_tensor(out=G2[:, gs], in0=PM[:, gs],
                                in1=expUab[:, gs], op=AL.mult)
        nc.vector.tensor_reduce(
            out=lamr[:, gs],
            in_=G2[:, gs].rearrange("p c r m -> p c (r m)"),
            axis=AX.X, op=AL.add)
    lam = wpool.tile([P, KC], f32, tag="lam")
    nc.vector.tensor_tensor(out=lam[:], in0=lamr[:], in1=grows[:, :, 20],
                            op=AL.add)
    loglam = wpool.tile([P, KC], f32, tag="loglam")
    nc.scalar.activation(loglam[:], lam[:], AF.Ln)

    # ---- combine and reduce ----
    per_event = wpool.tile([P, KC], f32, tag="per_event")
    nc.vector.tensor_tensor(out=per_event[:], in0=loglam[:], in1=negtot[:],
                            op=AL.add)
    colsum = wpool.tile([P, 1], f32, tag="colsum")
    nc.vector.tensor_reduce(out=colsum[:], in_=per_event[:], axis=AX.X,
                            op=AL.add)
    totp = ps.tile([1, 1], f32, tag="totp")
    nc.tensor.matmul(totp[:], ones_col[:], colsum[:], start=True, stop=True)
    final = spool.tile([1, 1], f32, tag="final")
    nc.vector.tensor_tensor(out=final[:], in0=totp[:], in1=negconst,
                            op=AL.add)
    nc.sync.dma_start(out=out_ap, in_=final[:])


_CACHE = {}


def _build(Tval: float):
    key = float(Tval)
    if key in _CACHE:
        return _CACHE[key]
    nc = bacc.Bacc("TRN2", target_bir_lowering=False, debug=False)
    ins = {}
    for name, (shape, dt) in INPUTS.items():
        ins[name] = nc.dram_tensor(name, list(shape), dt,
                                   kind="ExternalInput").ap()
    out_ap = nc.dram_tensor("out", [1, 1], f32, kind="ExternalOutput").ap()
    with tile.TileContext(nc) as tc:
        with ExitStack() as ctx:
            _body(ctx, tc, ins, out_ap, Tval)
    nc.compile()
    _CACHE[key] = (nc, ins, out_ap)
    return _CACHE[key]


def host_prep(mu_raw, log_alpha, log_beta, Tval):
    """O(D^2) parameter transforms in float64 -> float32."""
    mu = np.log1p(np.exp(np.float64(mu_raw))).astype(np.float32)
    al = np.log1p(np.exp(np.float64(log_alpha))).astype(np.float32)
    be = np.log1p(np.exp(np.float64(log_beta))).astype(np.float32)
    ab = (al * be).astype(np.float32)

    pack_bf = np.zeros((P, 588), dtype=ml_dtypes.bfloat16)
    pack_bf[:, 0:128] = np.triu(np.ones((P, P), dtype=np.float32))
    pack_bf[:, 128:228] = np.broadcast_to(ab.reshape(-1), (P, RM))
    pack_bf[:, 228:328] = np.broadcast_to(al.T.reshape(-1), (P, RM))
    pack_bf[:, 328:428] = np.broadcast_to(be.T.reshape(-1), (P, RM))
    pack_bf[:, 428:588] = np.tile(np.arange(D, dtype=np.float32), KC)[None, :]

    pf_const = np.zeros((P, 285), dtype=np.float32)
    pf_const[:, 48:148] = np.broadcast_to(be.reshape(-1), (P, RM))
    pf_const[:RM, 168:268] = np.eye(RM, dtype=np.float32)
    pf_const[0, 284] = np.float32(-Tval * mu.astype(np.float64).sum())

    tabs = np.zeros((D, 23), dtype=np.float32)
    tabs[:, 0:10] = be.T
    tabs[:, 10:20] = al.T
    tabs[:, 20] = mu - np.diag(ab)
    asum = al.sum(axis=0)
    asum_hi = asum.astype(ml_dtypes.bfloat16).astype(np.float32)
    tabs[:, 21] = asum_hi
    tabs[:, 22] = asum - asum_hi
    return pack_bf, pf_const, be, tabs


def make_in_maps(time_points, event_types, mu_raw, log_alpha, log_beta, T):
    Tval = float(np.asarray(T))
    tp = np.asarray(time_points, dtype=np.float32)
    et = np.asarray(event_types).astype(np.float32)
    pack_bf, pf_const, be, tabs = host_prep(
        np.asarray(mu_raw), np.asarray(log_alpha), np.asarray(log_beta), Tval)
    in_maps = []
    for b in range(B):
        ts = tp[b, ::P]                       # [16] chunk reference times
        dtb = np.zeros(KC, dtype=np.float32)
        dtb[:-1] = ts[1:] - ts[:-1]
        pack_f32 = pf_const.copy()
        pack_f32[:, 0:16] = tp[b].reshape(KC, P).T
        pack_f32[:, 16:32] = et[b].reshape(KC, P).T
        pack_f32[:, 32:48] = ts[None, :]
        pack_f32[:RM, 268:284] = be.reshape(-1)[:, None] * dtb[None, :]
        oht = np.zeros((D, N + 23), dtype=ml_dtypes.bfloat16)
        oht[:, 0:N] = (et[b][None, :] == np.arange(D, dtype=np.float32)[:, None])
        oht[:, N:N + 23] = tabs
        in_maps.append({"pack_f32": pack_f32, "pack_bf": pack_bf, "oht": oht})
    return in_maps, Tval


def kernel(time_points, event_types, mu_raw, log_alpha, log_beta, T):
    in_maps, Tval = make_in_maps(time_points, event_types, mu_raw,
                                 log_alpha, log_beta, T)
    nc, _, _ = _build(Tval)
    res = run_bass_kernel_spmd(nc, in_maps, list(range(B))).results
    out = np.array([res[b]["out"][0, 0] for b in range(B)], dtype=np.float32)
    return out
